# revision 1
# baseline (speedup 1.0000x reference)
"""Trainium2 Bass kernel for Derivative1D: y[:, i, :] = x[:, i+1, :] - x[:, i, :].

Full input x: [64, 16384, 32] f32; full output y: [64, 16383, 32] f32.
Sharding: pure data parallel over batch — 8 batches per core on 8 cores.

Layout (per core): each batch's (L, C) block is a contiguous stream of
L*C = 524288 f32, and the stencil in flat space is
y_flat[j] = x_flat[j+32] - x_flat[j] (shift by exactly C = 32 elements).
Batches are processed in fused groups of 4 because the fused output,
4*(L-1)*C = 2097024 = 128 * 16383, splits perfectly across 128 SBUF
partitions: partition p owns output elements [p*16383, (p+1)*16383) of the
group's output stream, and batch boundaries land exactly at partitions
32/64/96 (524256 = 32*16383).  Partition p = 32*q + i then needs input
x[batch q][i*16383 : i*16383 + 16383 + 32] — the final partition's window
ends exactly at the end of the batch, so the 32-element halo never reads
out of bounds anywhere.

Each group is processed in free-dim chunks: one [128, Fc+32] load (a 3D
access pattern over (window, batch-in-group, element)), one DVE subtract of
the two 32-shifted views, and the store split into two free-dim half
DMAs.  Loads use the sync HWDGE ring, which spreads a DMA's descriptors
across SDMA engines by its outermost access-pattern dim (kept at 32 here).
Stores use SWDGE (gpsimd): a single SBUF->HBM dma_start lands on ONE SDMA
engine (~27 GB/s), so the 2-way split (16 store DMAs total, one per
engine) plus multi-buffered pipelining engages all 16 engines at minimal
descriptor-emission cost.
"""

import sys

if "/opt/trn_rl_repo" not in sys.path:
    sys.path.insert(0, "/opt/trn_rl_repo")

import numpy as np

import concourse.bass as bass
import concourse.tile as tile
from concourse import bacc, mybir

B, L, C = 64, 16384, 32
NCORES = 8
BS = B // NCORES            # 8 batches per core
NF = L * C                  # 524288 flat input elements per batch
OF = (L - 1) * C            # 524256 flat output elements per batch
P = 128                     # SBUF partitions
H = C                       # halo: shift distance in flat space
G = 4                       # batches fused per group
NGROUP = BS // G            # 2 groups per core
FP = OF // 32               # 16383 output elements per partition per group
PB = P // G                 # 32 partitions per batch within a group
NCHUNK = 4                  # free-dim chunks per group
FC = 4096                   # chunk size (last chunk is 4095)
NSLICE = 2                  # free-dim store slices per chunk


def build_nc(repeat: int = 1, in_bufs: int = 6, out_bufs: int = 6):
    """Build the per-core Bass/Tile program (same program on all 8 cores)."""
    nc = bacc.Bacc(
        "TRN2",
        target_bir_lowering=False,
        debug=False,
        num_devices=NCORES,
        enable_partition_id=False,
    )
    x = nc.dram_tensor("x", [BS, L, C], mybir.dt.float32, kind="ExternalInput")
    y = nc.dram_tensor("y", [BS, L - 1, C], mybir.dt.float32, kind="ExternalOutput")

    with tile.TileContext(nc) as tc:
        with (
            tc.tile_pool(name="xin", bufs=in_bufs) as xin,
            tc.tile_pool(name="yout", bufs=out_bufs) as yout,
        ):
            for _ in range(repeat):
                for g in range(NGROUP):
                    for c in range(NCHUNK):
                        fc = FP - c * FC if c == NCHUNK - 1 else FC  # 4096/4095
                        t = xin.tile([P, FC + H], mybir.dt.float32)
                        # Interleaved partition layout: partition p holds
                        # window pin = p//4 of batch q = p%4.  The outermost
                        # access-pattern dim (32 windows) is what HWDGE
                        # round-robins across SDMA engines, so keep it large.
                        nc.sync.dma_start(
                            t[:, 0 : fc + H],
                            bass.AP(
                                x,
                                g * G * NF + c * FC,
                                [[FP, PB], [NF, G], [1, fc + H]],
                            ),
                        )
                        o = yout.tile([P, FC], mybir.dt.float32)
                        nc.vector.tensor_sub(
                            o[:, 0:fc], t[:, H : fc + H], t[:, 0:fc]
                        )
                        # Store in free-dim halves: each SWDGE dma_start
                        # lands on one SDMA engine; 16 slices total = exactly
                        # one per engine in the drain, at half the SWDGE
                        # descriptor-emission cost of finer slicing.
                        fs = (fc + NSLICE - 1) // NSLICE
                        for j in range(NSLICE):
                            fj = min(fs, fc - j * fs)
                            nc.gpsimd.dma_start(
                                bass.AP(
                                    y,
                                    g * G * OF + c * FC + j * fs,
                                    [[FP, PB], [OF, G], [1, fj]],
                                ),
                                o[:, j * fs : j * fs + fj],
                            )

    nc.compile()
    return nc


_NC_CACHE = {}


def _get_nc(repeat: int = 1):
    if repeat not in _NC_CACHE:
        _NC_CACHE[repeat] = build_nc(repeat)
    return _NC_CACHE[repeat]


def kernel(**inputs: np.ndarray) -> np.ndarray:
    x = np.ascontiguousarray(inputs["x"], dtype=np.float32)
    assert x.shape == (B, L, C), x.shape

    from concourse.bass_utils import run_bass_kernel_spmd

    nc = _get_nc()
    in_maps = [
        {"x": np.ascontiguousarray(x[c * BS : (c + 1) * BS])} for c in range(NCORES)
    ]
    try:
        res = run_bass_kernel_spmd(nc, in_maps, core_ids=list(range(NCORES)))
    except Exception:
        # A cold terminal can fail its very first execution transiently;
        # one retry has always succeeded.
        res = run_bass_kernel_spmd(nc, in_maps, core_ids=list(range(NCORES)))
    return np.concatenate([r["y"] for r in res.results], axis=0)



# revision 2
# speedup vs baseline: 1.0193x; 1.0193x over previous
"""Trainium2 Bass kernel for Derivative1D: y[:, i, :] = x[:, i+1, :] - x[:, i, :].

Full input x: [64, 16384, 32] f32; full output y: [64, 16383, 32] f32.
Sharding: pure data parallel over batch — 8 batches per core on 8 cores.

Layout (per core): each batch's (L, C) block is a contiguous stream of
L*C = 524288 f32, and the stencil in flat space is
y_flat[j] = x_flat[j+32] - x_flat[j] (shift by exactly C = 32 elements).
Batches are processed in fused groups of 4 because the fused output,
4*(L-1)*C = 2097024 = 128 * 16383, splits perfectly across 128 SBUF
partitions: partition p owns output elements [p*16383, (p+1)*16383) of the
group's output stream, and batch boundaries land exactly at partitions
32/64/96 (524256 = 32*16383).  Partition p = 32*q + i then needs input
x[batch q][i*16383 : i*16383 + 16383 + 32] — the final partition's window
ends exactly at the end of the batch, so the 32-element halo never reads
out of bounds anywhere.

DMA strategy (v2): loads on the SP HWDGE ring (nc.sync), stores on the
ACT HWDGE ring (nc.scalar).  Both rings spread one dma_start across all
16 SDMA engines, and each engine round-robins between the two rings at
packet granularity, so load and store streams interleave at full fabric
width with no software descriptor generation.  This avoids the SWDGE
(gpsimd) store path entirely: fp32 tensor_tensor on DVE holds the shared
SBUF port pair for the whole op, which locks the GPSIMD Q7 out of writing
SWDGE descriptors and stalls stores behind compute.  The final chunks
taper geometrically so the post-last-load tail (sub + store of the last
chunk) is ~1 us instead of ~10.
"""

import sys

if "/opt/trn_rl_repo" not in sys.path:
    sys.path.insert(0, "/opt/trn_rl_repo")

import numpy as np

import concourse.bass as bass
import concourse.tile as tile
from concourse import bacc, mybir

B, L, C = 64, 16384, 32
NCORES = 8
BS = B // NCORES            # 8 batches per core
NF = L * C                  # 524288 flat input elements per batch
OF = (L - 1) * C            # 524256 flat output elements per batch
P = 128                     # SBUF partitions
H = C                       # halo: shift distance in flat space
G = 4                       # batches fused per group
NGROUP = BS // G            # 2 groups per core
FP = OF // 32               # 16383 output elements per partition per group
PB = P // G                 # 32 partitions per batch within a group

# Free-dim chunk schedule per group.  Sum of each list is FP = 16383.
# The last group tapers so the tail after the final load is tiny.
CHUNKS = [
    [4096, 4096, 4096, 4095],
    [4096, 4096, 4096, 2047, 1024, 512, 256, 128, 128],
]
FCMAX = 4096


def build_nc(repeat: int = 1, in_bufs: int = 6, out_bufs: int = 6):
    """Build the per-core Bass/Tile program (same program on all 8 cores)."""
    nc = bacc.Bacc(
        "TRN2",
        target_bir_lowering=False,
        debug=False,
        num_devices=NCORES,
        enable_partition_id=False,
    )
    x = nc.dram_tensor("x", [BS, L, C], mybir.dt.float32, kind="ExternalInput")
    y = nc.dram_tensor("y", [BS, L - 1, C], mybir.dt.float32, kind="ExternalOutput")

    with tile.TileContext(nc) as tc:
        with (
            tc.tile_pool(name="xin", bufs=in_bufs) as xin,
            tc.tile_pool(name="yout", bufs=out_bufs) as yout,
        ):
            for _ in range(repeat):
                for g in range(NGROUP):
                    off = 0
                    for fc in CHUNKS[g]:
                        t = xin.tile([P, FCMAX + H], mybir.dt.float32)
                        # Interleaved partition layout: partition p holds
                        # window pin = p//4 of batch q = p%4.
                        nc.sync.dma_start(
                            t[:, 0 : fc + H],
                            bass.AP(
                                x,
                                g * G * NF + off,
                                [[FP, PB], [NF, G], [1, fc + H]],
                            ),
                        )
                        o = yout.tile([P, FCMAX], mybir.dt.float32)
                        nc.vector.tensor_sub(
                            o[:, 0:fc], t[:, H : fc + H], t[:, 0:fc]
                        )
                        nc.scalar.dma_start(
                            bass.AP(
                                y,
                                g * G * OF + off,
                                [[FP, PB], [OF, G], [1, fc]],
                            ),
                            o[:, 0:fc],
                        )
                        off += fc

    nc.compile()
    return nc


_NC_CACHE = {}


def _get_nc(repeat: int = 1):
    if repeat not in _NC_CACHE:
        _NC_CACHE[repeat] = build_nc(repeat)
    return _NC_CACHE[repeat]


def kernel(**inputs: np.ndarray) -> np.ndarray:
    x = np.ascontiguousarray(inputs["x"], dtype=np.float32)
    assert x.shape == (B, L, C), x.shape

    from concourse.bass_utils import run_bass_kernel_spmd

    nc = _get_nc()
    in_maps = [
        {"x": np.ascontiguousarray(x[c * BS : (c + 1) * BS])} for c in range(NCORES)
    ]
    try:
        res = run_bass_kernel_spmd(nc, in_maps, core_ids=list(range(NCORES)))
    except Exception:
        # A cold terminal can fail its very first execution transiently;
        # one retry has always succeeded.
        res = run_bass_kernel_spmd(nc, in_maps, core_ids=list(range(NCORES)))
    return np.concatenate([r["y"] for r in res.results], axis=0)
